# revision 5
# baseline (speedup 1.0000x reference)
# Trainium2 Bass kernel for nn_CrossAttention (B=2, Nq=4096, Nk=2048, D=128,
# Dv=768, H=4, hd=32).
#
# Sharding: data-parallel over (B x Nq-blocks): core c handles batch c//4,
# query rows (c%4)*1024 .. +1024. K/V/weights replicated per core.
#
# Math (host-folded):
#   qn = (q * rstd_q) @ WqT_eff + bq/sqrt(hd)   with WqT_eff = diag(rms_q_w) Wq^T / sqrt(hd)
#   kn = (k * rstd_k) @ WkT_eff + bk            with WkT_eff = diag(rms_k_w) Wk^T
#   S_h = qn_h kn_h^T  (scale already folded into q side)
#   A   = sum_h exp(S_h) / rowsum_h(exp S_h)    (no max subtraction: |S| < 8)
#   out = A @ (0.25 * V)
#
# Pipeline: ACT(exp) is the pacer (~10.6us/qtile floor). lag-1 structure:
# per head, exp -> (rsum, recip, apath increment) on DVE immediately; PV of
# qtile qc-1 interleaved into PE gaps between score matmuls of qc.
import numpy as np

B, NQ, NK, D, DV = 2, 4096, 2048, 128, 768
H, HD = 4, 32
N_CORES = 8
NQC = NQ * B // N_CORES  # 1024 queries per core
NQT = NQC // 128  # 8 query tiles per core
NKT = NK // 128  # 16 key tiles
RMS_EPS = 1.1920929e-07
DVH = DV // 2  # 384: one PSUM bank per dv-half

_CACHE = {}


def _build_nc():
    import concourse.bacc as bacc
    import concourse.mybir as mybir
    import concourse.tile as tile

    fp32 = mybir.dt.float32
    f16 = mybir.dt.float16

    nc = bacc.Bacc("TRN2", target_bir_lowering=False, debug=False)

    q_d = nc.dram_tensor("q", [NQC, D], f16, kind="ExternalInput").ap()
    k_d = nc.dram_tensor("k", [NK, D], f16, kind="ExternalInput").ap()
    v_d = nc.dram_tensor("v", [NK, DV], f16, kind="ExternalInput").ap()
    wq_d = nc.dram_tensor("wqt", [D, D], f16, kind="ExternalInput").ap()
    wk_d = nc.dram_tensor("wkt", [D, D], f16, kind="ExternalInput").ap()
    bq_d = nc.dram_tensor("bqe", [D], fp32, kind="ExternalInput").ap()
    bk_d = nc.dram_tensor("bke", [D], fp32, kind="ExternalInput").ap()
    o_d = nc.dram_tensor("o", [NQC, DV], f16, kind="ExternalOutput").ap()

    with tile.TileContext(nc) as tc:
        _tile_kernel(tc, o_d, q_d, k_d, v_d, wq_d, wk_d, bq_d, bk_d)
    nc.compile()
    return nc


def _tile_kernel(tc, o_d, q_d, k_d, v_d, wq_d, wk_d, bq_d, bk_d):
    from contextlib import ExitStack

    import concourse.mybir as mybir

    nc = tc.nc
    fp32 = mybir.dt.float32
    f16 = mybir.dt.float16
    AF = mybir.ActivationFunctionType
    OP = mybir.AluOpType
    AX = mybir.AxisListType

    ctx = ExitStack()
    with ctx:
        singles = ctx.enter_context(tc.tile_pool(name="singles", bufs=1))

        # --- input loads. p-outer layouts: partition p reads a CONTIGUOUS
        # run of rows -> large DMA descriptors. Token order inside the core
        # becomes p-outer-permuted; consistent everywhere (kT<->V, qT<->out).
        # k in 2 halves so stats can start at half ready; q likewise.
        kx_sb = singles.tile([128, NKT, D], f16)
        nc.sync.dma_start(
            out=kx_sb[:, 0:8, :],
            in_=k_d.rearrange("(p c) d -> p c d", c=NKT)[:, 0:8, :],
        )
        nc.sync.dma_start(
            out=kx_sb[:, 8:16, :],
            in_=k_d.rearrange("(p c) d -> p c d", c=NKT)[:, 8:16, :],
        )
        qx_sb = singles.tile([128, NQT, D], f16)
        nc.sync.dma_start(
            out=qx_sb[:, 0:4, :],
            in_=q_d.rearrange("(p c) d -> p c d", c=NQT)[:, 0:4, :],
        )
        nc.sync.dma_start(
            out=qx_sb[:, 4:8, :],
            in_=q_d.rearrange("(p c) d -> p c d", c=NQT)[:, 4:8, :],
        )
        # v on sync behind k/q; only PV needs it (~18us in)
        v_sb = singles.tile([128, NKT, DV], f16)
        nc.sync.dma_start(out=v_sb, in_=v_d.rearrange("(p c) d -> p c d", c=NKT))
        # weights/biases via gpsimd SWDGE (keeps sync queue clear for
        # transposes and ACT free for sqrt/exp); k-side first.
        wk_sb = singles.tile([128, D], f16)
        nc.gpsimd.dma_start(out=wk_sb, in_=wk_d)
        bk_sb = singles.tile([128, 1], fp32)
        nc.gpsimd.dma_start(out=bk_sb, in_=bk_d[:, None])
        wq_sb = singles.tile([128, D], f16)
        nc.gpsimd.dma_start(out=wq_sb, in_=wq_d)
        bq_sb = singles.tile([128, 1], fp32)
        nc.gpsimd.dma_start(out=bq_sb, in_=bq_d[:, None])

        eps_sb = singles.tile([128, 1], fp32)
        nc.vector.memset(eps_sb, RMS_EPS)
        warmsrc = singles.tile([128, 512], f16)
        nc.vector.memset(warmsrc, 0.125)

        kxT = singles.tile([128, NK], f16)  # normalized, transposed [d, tok]
        qxT = singles.tile([128, NQC], f16)
        kT = singles.tile([128, NK], f16)  # projected (head h rows 32h..32h+31)
        qT = singles.tile([128, NQC], f16)

        # ---- preamble: RMSNorm + transpose + projections (k first) ----
        with (
            tc.tile_pool(name="pre", bufs=1) as pre,
            tc.tile_pool(name="prepsum", bufs=2, space="PSUM") as prepsum,
        ):
            # PE warm-up at t~0: >=3.4us of continuous matmul busy fires the
            # HAM un-throttle (1.2 -> 2.4 GHz) before proj/scores arrive.
            warm = prepsum.tile([128, 512], fp32, tag="warm", bufs=1)
            for _ in range(10):
                nc.tensor.matmul(warm, lhsT=warmsrc[:, 0:128], rhs=warmsrc,
                                 start=True, stop=True)

            def mk_side(nt, tag):
                ssq = pre.tile([128, nt], fp32, tag=f"ssq{tag}", name=f"ssq{tag}")
                sd = pre.tile([128, nt], fp32, tag=f"sd{tag}", name=f"sd{tag}")
                rstd = pre.tile([128, nt], fp32, tag=f"rstd{tag}", name=f"rstd{tag}")
                xn = pre.tile([128, nt, D], f16, tag=f"xn{tag}", name=f"xn{tag}")
                return ssq, sd, rstd, xn

            def stats_half(x_sb, side, hh, n8=8):
                ssq, sd, rstd, xn = side
                sl = slice(hh * n8, (hh + 1) * n8)
                hsl = slice(hh * n8, (hh + 1) * n8)
                sq = pre.tile([128, n8, D], f16, tag="sqh", bufs=2,
                              name=f"sq_{id(side)}_{hh}")
                nc.vector.tensor_mul(sq, x_sb[:, sl, :], x_sb[:, sl, :])
                nc.vector.tensor_reduce(ssq[:, hsl, None], sq, AX.X, OP.add)
                nc.scalar.activation(
                    sd[:, hsl], ssq[:, hsl], AF.Sqrt, bias=eps_sb, scale=1.0 / D
                )
                nc.vector.reciprocal(rstd[:, hsl], sd[:, hsl])
                for t in range(hh * n8, (hh + 1) * n8):
                    nc.vector.tensor_scalar_mul(
                        xn[:, t, :], x_sb[:, t, :], rstd[:, t : t + 1]
                    )

            def transpose_half(xn, xT, hh, n8=8):
                w = n8 * 128
                nc.sync.dma_start_transpose(
                    out=xT[:, hh * w : (hh + 1) * w].rearrange(
                        "p (c j) -> p c j", j=128
                    ),
                    in_=xn[:, hh * n8 : (hh + 1) * n8, :].rearrange(
                        "p c j -> p (c j)"
                    ),
                )

            _pj = [0]

            def proj(xT, w_sb, b_sb, dst, j):
                # bias-add eviction on ACT (Identity w/ per-partition bias);
                # keeps DVE free for stats of the next half.
                _pj[0] += 1
                pp = prepsum.tile([128, 512], fp32, tag="proj", name=f"pp{_pj[0]}")
                nc.tensor.matmul(
                    pp, lhsT=w_sb, rhs=xT[:, j * 512 : (j + 1) * 512],
                    start=True, stop=True,
                )
                nc.scalar.activation(
                    dst[:, j * 512 : (j + 1) * 512], pp, AF.Identity, bias=b_sb
                )

            kside = mk_side(NKT, "k")
            qside = mk_side(NQT, "q")
            kxn, qxn = kside[3], qside[3]
            stats_half(kx_sb, kside, 0)
            transpose_half(kxn, kxT, 0)
            proj(kxT, wk_sb, bk_sb, kT, 0)
            proj(kxT, wk_sb, bk_sb, kT, 1)
            stats_half(qx_sb, qside, 0, n8=4)
            stats_half(kx_sb, kside, 1)
            # preload the exp table set now that all Sqrt calls are emitted;
            # runs in the preamble shadow instead of delaying the first exp.
            dummy_exp = pre.tile([128, 1], fp32, tag="dexp", bufs=1)
            nc.scalar.activation(dummy_exp, eps_sb, AF.Exp)
            transpose_half(qxn, qxT, 0, n8=4)
            proj(qxT, wq_sb, bq_sb, qT, 0)
            transpose_half(kxn, kxT, 1)
            proj(kxT, wk_sb, bk_sb, kT, 2)
            proj(kxT, wk_sb, bk_sb, kT, 3)
            stats_half(qx_sb, qside, 1, n8=4)
            transpose_half(qxn, qxT, 1, n8=4)
            proj(qxT, wq_sb, bq_sb, qT, 1)

        # ---- main loop: lag-1 software pipeline ----
        with (
            tc.tile_pool(name="spsum", bufs=3, space="PSUM") as spool,
            tc.tile_pool(name="opsum", bufs=1, space="PSUM") as opool,
            tc.tile_pool(name="pwork", bufs=2) as pwork,
            tc.tile_pool(name="awork", bufs=2) as awork,
            tc.tile_pool(name="twork", bufs=2) as twork,
            tc.tile_pool(name="owork", bufs=2) as owork,
            tc.tile_pool(name="small", bufs=2) as small,
        ):
            st = {}
            o_view = o_d.rearrange("(j c) d -> c j d", c=NQT)

            def emit_scores(qc, h, half):
                # S chunk [128,1024] fp32 (2 PSUM banks) via 2 FD-512 matmuls
                qsl = slice(qc * 128, (qc + 1) * 128)
                S = spool.tile([128, 1024], fp32, tag="S",
                               name=f"S_{qc}_{h}_{half}")
                for kc in range(2):
                    ko = half * 1024 + kc * 512
                    nc.tensor.matmul(
                        S[:, kc * 512 : (kc + 1) * 512],
                        lhsT=qT[32 * h : 32 * (h + 1), qsl],
                        rhs=kT[32 * h : 32 * (h + 1), ko : ko + 512],
                        start=True, stop=True,
                        tile_position=(32 * h, 0),
                    )
                return S

            def emit_exp(qc, h, half, S):
                s = st[qc]
                psl = s["P"][:, h, half * 1024 : (half + 1) * 1024]
                nc.scalar.activation(
                    psl, S, AF.Exp, accum_out=s["racc"][:, h, half : half + 1]
                )

            def emit_apath(qc, h):
                # after exp(qc,h,1): crec_h then A += crec_h * P_h (2 halves)
                s = st[qc]
                rsum = small.tile([128, 1], fp32, tag=f"rs{h}", name=f"rs_{qc}_{h}")
                nc.vector.tensor_scalar(
                    rsum, s["racc"][:, h, 0:1], s["racc"][:, h, 1:2], None, OP.add
                )
                crec = small.tile([128, 1], fp32, tag=f"cr{h}", name=f"cr_{qc}_{h}")
                nc.vector.reciprocal(crec, rsum)
                A, P = s["A"], s["P"]
                for half in range(2):
                    hsl = slice(half * 1024, (half + 1) * 1024)
                    if h == 0:
                        nc.vector.tensor_scalar_mul(A[:, hsl], P[:, 0, hsl], crec)
                    else:
                        t = twork.tile([128, 1024], f16, tag=f"t{half}",
                                       name=f"t_{qc}_{h}_{half}")
                        nc.vector.tensor_scalar_mul(t, P[:, h, hsl], crec)
                        nc.vector.tensor_add(A[:, hsl], A[:, hsl], t)

            def emit_at(qc, half):
                s = st[qc]
                ksl = slice(half * 1024, (half + 1) * 1024)
                nc.sync.dma_start_transpose(
                    out=s["AT"][:, ksl].rearrange("p (c j) -> p c j", j=128),
                    in_=s["A"][:, ksl],
                )

            def emit_pv(qc, dvh, kcs):
                s = st[qc]
                key = f"O{dvh}"
                if key not in s:
                    s[key] = opool.tile([128, DVH], fp32, tag=key,
                                        name=f"{key}_{qc}")
                O, AT = s[key], s["AT"]
                for kc in kcs:
                    nc.tensor.matmul(
                        O,
                        lhsT=AT[:, kc * 128 : (kc + 1) * 128],
                        rhs=v_sb[:, kc, dvh * DVH : (dvh + 1) * DVH],
                        start=kc == 0,
                        stop=kc == NKT - 1,
                    )

            def emit_evict_store(qc, dvh):
                s = st[qc]
                if "osb" not in s:
                    s["osb"] = owork.tile([128, DV], f16, tag="osb",
                                          name=f"osb_{qc}")
                sl = slice(dvh * DVH, (dvh + 1) * DVH)
                nc.vector.tensor_copy(s["osb"][:, sl], s[f"O{dvh}"])
                ov = o_view[qc]
                nc.sync.dma_start(out=ov[:, :, sl] if len(ov.shape) == 3
                                  else ov[:, sl], in_=s["osb"][:, sl])

            def emit_fill(qc, n, dvh=0):
                # keep-warm filler matmuls into a dummy O-tagged psum tile
                dmy = opool.tile([128, DVH], fp32, tag=f"O{dvh}",
                                 name=f"dmy{dvh}_{qc}")
                for _ in range(n):
                    nc.tensor.matmul(dmy, lhsT=kT[:, 0:128], rhs=kT[:, 0:DVH],
                                     start=True, stop=True)

            for qc in range(NQT):
                P = pwork.tile([128, H, NK], f16, tag="P", name=f"P_{qc}")
                racc = small.tile([128, H, 2], fp32, tag="racc",
                                  name=f"racc_{qc}")
                A = awork.tile([128, NK], f16, tag="A", name=f"A_{qc}")
                AT = awork.tile([128, NK], f16, tag="AT", name=f"AT_{qc}")
                st[qc] = {"P": P, "racc": racc, "A": A, "AT": AT}
                for h in range(H):
                    if qc >= 1:
                        if h == 0:
                            emit_pv(qc - 1, 0, range(0, 8))
                        elif h == 1:
                            emit_pv(qc - 1, 0, range(8, NKT))
                        elif h == 2:
                            emit_pv(qc - 1, 1, range(0, 8))
                        elif h == 3:
                            emit_pv(qc - 1, 1, range(8, NKT))
                    elif h > 0:
                        emit_fill(qc, 8, dvh=h % 2)
                    S0 = emit_scores(qc, h, 0)
                    S1 = emit_scores(qc, h, 1)
                    emit_exp(qc, h, 0, S0)
                    emit_exp(qc, h, 1, S1)
                    if qc >= 1:
                        if h == 1:
                            emit_evict_store(qc - 1, 0)
                        elif h == 3:
                            emit_evict_store(qc - 1, 1)
                    emit_apath(qc, h)
                emit_at(qc, 0)
                emit_at(qc, 1)

            # ---- drain: PV of the last qtile ----
            qc = NQT - 1
            emit_pv(qc, 0, range(0, 8))
            emit_pv(qc, 1, range(0, 8))
            emit_pv(qc, 0, range(8, NKT))
            emit_pv(qc, 1, range(8, NKT))
            emit_evict_store(qc, 0)
            emit_evict_store(qc, 1)


def _get_nc():
    if "nc" not in _CACHE:
        _CACHE["nc"] = _build_nc()
    return _CACHE["nc"]


def _host_prep(query, key, value, rms_q_w, rms_k_w, Wq, Wk, bq, bk):
    s = np.sqrt(float(HD))
    wqt = (rms_q_w[:, None] * Wq.T / s).astype(np.float16)
    wkt = (rms_k_w[:, None] * Wk.T).astype(np.float16)
    bqe = (bq / s).astype(np.float32)
    bke = bk.astype(np.float32)
    vq = (0.25 * value).astype(np.float16)  # [B, NK, DV]
    in_maps = []
    nq_blk = NQ // (N_CORES // B)  # 1024
    for c in range(N_CORES):
        b, qi = divmod(c, N_CORES // B)
        in_maps.append(
            {
                "q": np.ascontiguousarray(
                    query[b, qi * nq_blk : (qi + 1) * nq_blk]
                ).astype(np.float16),
                "k": np.ascontiguousarray(key[b]).astype(np.float16),
                "v": np.ascontiguousarray(vq[b]),
                "wqt": wqt,
                "wkt": wkt,
                "bqe": bqe,
                "bke": bke,
            }
        )
    return in_maps


def kernel(query, key, value, rms_q_w, rms_k_w, Wq, Wk, bq, bk, _trace=False):
    from concourse import bass_utils

    in_maps = _host_prep(
        np.asarray(query), np.asarray(key), np.asarray(value),
        np.asarray(rms_q_w), np.asarray(rms_k_w),
        np.asarray(Wq), np.asarray(Wk), np.asarray(bq), np.asarray(bk),
    )
    nc = _get_nc()
    res = bass_utils.run_bass_kernel_spmd(
        nc, in_maps, core_ids=list(range(N_CORES)), trace=_trace
    )
    _CACHE["last_results"] = res
    outs = [np.asarray(r["o"], dtype=np.float32) for r in res.results]
    nq_blk = NQ // (N_CORES // B)
    out = np.empty((B, NQ, DV), dtype=np.float32)
    for c in range(N_CORES):
        b, qi = divmod(c, N_CORES // B)
        out[b, qi * nq_blk : (qi + 1) * nq_blk] = outs[c]
    return out


# revision 8
# speedup vs baseline: 1.1324x; 1.1324x over previous
# Trainium2 Bass kernel for nn_CrossAttention (B=2, Nq=4096, Nk=2048, D=128,
# Dv=768, H=4, hd=32).
#
# Sharding: data-parallel over (B x Nq-blocks): core c handles batch c//4,
# query rows (c%4)*1024 .. +1024. K/V/weights replicated per core.
#
# Math (host-folded):
#   qn = (q * rstd_q) @ WqT_eff + bq/sqrt(hd)   with WqT_eff = diag(rms_q_w) Wq^T / sqrt(hd)
#   kn = (k * rstd_k) @ WkT_eff + bk            with WkT_eff = diag(rms_k_w) Wk^T
#   S_h = qn_h kn_h^T  (scale already folded into q side)
#   A   = sum_h exp(S_h) / rowsum_h(exp S_h)    (no max subtraction: |S| < 8)
#   out = A @ (0.25 * V)
#
# Pipeline: ACT(exp) is the pacer (~10.6us/qtile floor). lag-1 structure:
# per head, exp -> (rsum, recip, apath increment) on DVE immediately; PV of
# qtile qc-1 interleaved into PE gaps between score matmuls of qc.
import numpy as np

B, NQ, NK, D, DV = 2, 4096, 2048, 128, 768
H, HD = 4, 32
N_CORES = 8
NQC = NQ * B // N_CORES  # 1024 queries per core
NQT = NQC // 128  # 8 query tiles per core
NKT = NK // 128  # 16 key tiles
RMS_EPS = 1.1920929e-07
DVH = DV // 2  # 384: one PSUM bank per dv-half

_CACHE = {}


def _build_nc():
    import concourse.bacc as bacc
    import concourse.mybir as mybir
    import concourse.tile as tile

    fp32 = mybir.dt.float32
    f16 = mybir.dt.float16

    nc = bacc.Bacc("TRN2", target_bir_lowering=False, debug=False)

    q_d = nc.dram_tensor("q", [NQC, D], f16, kind="ExternalInput").ap()
    k_d = nc.dram_tensor("k", [NK, D], f16, kind="ExternalInput").ap()
    v_d = nc.dram_tensor("v", [NK, DV], f16, kind="ExternalInput").ap()
    wq_d = nc.dram_tensor("wqt", [D, D], f16, kind="ExternalInput").ap()
    wk_d = nc.dram_tensor("wkt", [D, D], f16, kind="ExternalInput").ap()
    bq_d = nc.dram_tensor("bqe", [D], fp32, kind="ExternalInput").ap()
    bk_d = nc.dram_tensor("bke", [D], fp32, kind="ExternalInput").ap()
    o_d = nc.dram_tensor("o", [NQC, DV], f16, kind="ExternalOutput").ap()

    with tile.TileContext(nc) as tc:
        _tile_kernel(tc, o_d, q_d, k_d, v_d, wq_d, wk_d, bq_d, bk_d)
    nc.compile()
    return nc


def _tile_kernel(tc, o_d, q_d, k_d, v_d, wq_d, wk_d, bq_d, bk_d):
    from contextlib import ExitStack

    import concourse.mybir as mybir

    nc = tc.nc
    fp32 = mybir.dt.float32
    f16 = mybir.dt.float16
    AF = mybir.ActivationFunctionType
    OP = mybir.AluOpType
    AX = mybir.AxisListType

    ctx = ExitStack()
    with ctx:
        singles = ctx.enter_context(tc.tile_pool(name="singles", bufs=1))

        # --- input loads. p-outer layouts: partition p reads a CONTIGUOUS
        # run of rows -> large DMA descriptors. Token order inside the core
        # becomes p-outer-permuted; consistent everywhere (kT<->V, qT<->out).
        # k in 2 halves so stats can start at half ready; q likewise.
        kx_sb = singles.tile([128, NKT, D], f16)
        nc.sync.dma_start(
            out=kx_sb[:, 0:8, :],
            in_=k_d.rearrange("(p c) d -> p c d", c=NKT)[:, 0:8, :],
        )
        nc.sync.dma_start(
            out=kx_sb[:, 8:16, :],
            in_=k_d.rearrange("(p c) d -> p c d", c=NKT)[:, 8:16, :],
        )
        qx_sb = singles.tile([128, NQT, D], f16)
        nc.sync.dma_start(
            out=qx_sb[:, 0:4, :],
            in_=q_d.rearrange("(p c) d -> p c d", c=NQT)[:, 0:4, :],
        )
        nc.sync.dma_start(
            out=qx_sb[:, 4:8, :],
            in_=q_d.rearrange("(p c) d -> p c d", c=NQT)[:, 4:8, :],
        )
        # v on the scalar queue: one dispatch (~0.7us) while ACT is idle;
        # keeps the sync queue free for the kxT/qxT transposes. Only PV
        # needs v (~18us in).
        v_sb = singles.tile([128, NKT, DV], f16)
        nc.scalar.dma_start(out=v_sb, in_=v_d.rearrange("(p c) d -> p c d", c=NKT))
        # weights/biases via gpsimd SWDGE (keeps sync queue clear for
        # transposes and ACT free for sqrt/exp); k-side first.
        wk_sb = singles.tile([128, D], f16)
        nc.gpsimd.dma_start(out=wk_sb, in_=wk_d)
        bk_sb = singles.tile([128, 1], fp32)
        nc.gpsimd.dma_start(out=bk_sb, in_=bk_d[:, None])
        wq_sb = singles.tile([128, D], f16)
        nc.gpsimd.dma_start(out=wq_sb, in_=wq_d)
        bq_sb = singles.tile([128, 1], fp32)
        nc.gpsimd.dma_start(out=bq_sb, in_=bq_d[:, None])

        eps_sb = singles.tile([128, 1], fp32)
        nc.vector.memset(eps_sb, RMS_EPS)
        warmsrc = singles.tile([128, 512], f16)
        nc.vector.memset(warmsrc, 0.125)

        kxT = singles.tile([128, NK], f16)  # normalized, transposed [d, tok]
        qxT = singles.tile([128, NQC], f16)
        kT = singles.tile([128, NK], f16)  # projected (head h rows 32h..32h+31)
        qT = singles.tile([128, NQC], f16)

        # ---- preamble: RMSNorm + transpose + projections (k first) ----
        with (
            tc.tile_pool(name="pre", bufs=1) as pre,
            tc.tile_pool(name="prepsum", bufs=2, space="PSUM") as prepsum,
        ):
            # PE warm-up at t~0: >=3.4us of continuous matmul busy fires the
            # HAM un-throttle (1.2 -> 2.4 GHz) before proj/scores arrive.
            warm = prepsum.tile([128, 512], fp32, tag="warm", bufs=1)
            for _ in range(10):
                nc.tensor.matmul(warm, lhsT=warmsrc[:, 0:128], rhs=warmsrc,
                                 start=True, stop=True)

            def mk_side(nt, tag):
                ssq = pre.tile([128, nt], fp32, tag=f"ssq{tag}", name=f"ssq{tag}")
                sd = pre.tile([128, nt], fp32, tag=f"sd{tag}", name=f"sd{tag}")
                rstd = pre.tile([128, nt], fp32, tag=f"rstd{tag}", name=f"rstd{tag}")
                xn = pre.tile([128, nt, D], f16, tag=f"xn{tag}", name=f"xn{tag}")
                return ssq, sd, rstd, xn

            def stats_half(x_sb, side, hh, n8=8):
                ssq, sd, rstd, xn = side
                sl = slice(hh * n8, (hh + 1) * n8)
                hsl = slice(hh * n8, (hh + 1) * n8)
                sq = pre.tile([128, n8, D], f16, tag="sqh", bufs=2,
                              name=f"sq_{id(side)}_{hh}")
                nc.vector.tensor_mul(sq, x_sb[:, sl, :], x_sb[:, sl, :])
                nc.vector.tensor_reduce(ssq[:, hsl, None], sq, AX.X, OP.add)
                nc.scalar.activation(
                    sd[:, hsl], ssq[:, hsl], AF.Sqrt, bias=eps_sb, scale=1.0 / D
                )
                nc.vector.reciprocal(rstd[:, hsl], sd[:, hsl])
                for t in range(hh * n8, (hh + 1) * n8):
                    nc.vector.tensor_scalar_mul(
                        xn[:, t, :], x_sb[:, t, :], rstd[:, t : t + 1]
                    )

            def transpose_half(xn, xT, hh, n8=8):
                w = n8 * 128
                nc.sync.dma_start_transpose(
                    out=xT[:, hh * w : (hh + 1) * w].rearrange(
                        "p (c j) -> p c j", j=128
                    ),
                    in_=xn[:, hh * n8 : (hh + 1) * n8, :].rearrange(
                        "p c j -> p (c j)"
                    ),
                )

            _pj = [0]

            def proj(xT, w_sb, b_sb, dst, j):
                # bias-add eviction on ACT (Identity w/ per-partition bias);
                # keeps DVE free for stats of the next half.
                _pj[0] += 1
                pp = prepsum.tile([128, 512], fp32, tag="proj", name=f"pp{_pj[0]}")
                nc.tensor.matmul(
                    pp, lhsT=w_sb, rhs=xT[:, j * 512 : (j + 1) * 512],
                    start=True, stop=True,
                )
                nc.scalar.activation(
                    dst[:, j * 512 : (j + 1) * 512], pp, AF.Identity, bias=b_sb
                )

            kside = mk_side(NKT, "k")
            qside = mk_side(NQT, "q")
            kxn, qxn = kside[3], qside[3]
            stats_half(kx_sb, kside, 0)
            transpose_half(kxn, kxT, 0)
            proj(kxT, wk_sb, bk_sb, kT, 0)
            proj(kxT, wk_sb, bk_sb, kT, 1)
            stats_half(qx_sb, qside, 0, n8=4)
            stats_half(kx_sb, kside, 1)
            # preload the exp table set now that all Sqrt calls are emitted;
            # runs in the preamble shadow instead of delaying the first exp.
            dummy_exp = pre.tile([128, 1], fp32, tag="dexp", bufs=1)
            nc.scalar.activation(dummy_exp, eps_sb, AF.Exp)
            transpose_half(qxn, qxT, 0, n8=4)
            proj(qxT, wq_sb, bq_sb, qT, 0)
            transpose_half(kxn, kxT, 1)
            proj(kxT, wk_sb, bk_sb, kT, 2)
            proj(kxT, wk_sb, bk_sb, kT, 3)
            stats_half(qx_sb, qside, 1, n8=4)
            transpose_half(qxn, qxT, 1, n8=4)
            proj(qxT, wq_sb, bq_sb, qT, 1)

        # ---- main loop: lag-1 software pipeline ----
        with (
            tc.tile_pool(name="spsum", bufs=3, space="PSUM") as spool,
            tc.tile_pool(name="opsum", bufs=1, space="PSUM") as opool,
            tc.tile_pool(name="pwork", bufs=2) as pwork,
            tc.tile_pool(name="awork", bufs=2) as awork,
            tc.tile_pool(name="twork", bufs=2) as twork,
            tc.tile_pool(name="owork", bufs=2) as owork,
            tc.tile_pool(name="small", bufs=2) as small,
        ):
            st = {}
            o_view = o_d.rearrange("(j c) d -> c j d", c=NQT)

            def emit_scores(qc, h, half):
                # S chunk [128,1024] fp32 (2 PSUM banks) via 2 FD-512 matmuls
                qsl = slice(qc * 128, (qc + 1) * 128)
                S = spool.tile([128, 1024], fp32, tag="S",
                               name=f"S_{qc}_{h}_{half}")
                for kc in range(2):
                    ko = half * 1024 + kc * 512
                    nc.tensor.matmul(
                        S[:, kc * 512 : (kc + 1) * 512],
                        lhsT=qT[32 * h : 32 * (h + 1), qsl],
                        rhs=kT[32 * h : 32 * (h + 1), ko : ko + 512],
                        start=True, stop=True,
                        tile_position=(32 * h, 0),
                    )
                return S

            def emit_exp(qc, h, half, S):
                s = st[qc]
                psl = s["P"][:, h, half * 1024 : (half + 1) * 1024]
                nc.scalar.activation(
                    psl, S, AF.Exp, accum_out=s["racc"][:, h, half : half + 1]
                )

            def emit_at(qc, half):
                s = st[qc]
                ksl = slice(half * 1024, (half + 1) * 1024)
                nc.sync.dma_start_transpose(
                    out=s["AT"][:, ksl].rearrange("p (c j) -> p c j", j=128),
                    in_=s["A"][:, ksl],
                )

            def emit_apath(qc, h):
                # after exp(qc,h,1): crec_h then A += crec_h * P_h (2 halves);
                # for h==3 each completed A-half immediately launches its
                # transpose so PV(qc) can start early next qtile.
                s = st[qc]
                rsum = small.tile([128, 1], fp32, tag=f"rs{h}", name=f"rs_{qc}_{h}")
                nc.vector.tensor_scalar(
                    rsum, s["racc"][:, h, 0:1], s["racc"][:, h, 1:2], None, OP.add
                )
                crec = small.tile([128, 1], fp32, tag=f"cr{h}", name=f"cr_{qc}_{h}")
                nc.vector.reciprocal(crec, rsum)
                A, P = s["A"], s["P"]
                for half in range(2):
                    hsl = slice(half * 1024, (half + 1) * 1024)
                    if h == 0:
                        nc.vector.tensor_scalar_mul(A[:, hsl], P[:, 0, hsl], crec)
                    else:
                        t = twork.tile([128, 1024], f16, tag=f"t{half}",
                                       name=f"t_{qc}_{h}_{half}")
                        nc.vector.tensor_scalar_mul(t, P[:, h, hsl], crec)
                        nc.vector.tensor_add(A[:, hsl], A[:, hsl], t)
                    if h == 3:
                        emit_at(qc, half)

            def emit_pv(qc, dvh, kcs):
                s = st[qc]
                key = f"O{dvh}"
                if key not in s:
                    s[key] = opool.tile([128, DVH], fp32, tag=key,
                                        name=f"{key}_{qc}")
                O, AT = s[key], s["AT"]
                for kc in kcs:
                    nc.tensor.matmul(
                        O,
                        lhsT=AT[:, kc * 128 : (kc + 1) * 128],
                        rhs=v_sb[:, kc, dvh * DVH : (dvh + 1) * DVH],
                        start=kc == 0,
                        stop=kc == NKT - 1,
                    )

            def emit_evict_store(qc, dvh):
                s = st[qc]
                if "osb" not in s:
                    s["osb"] = owork.tile([128, DV], f16, tag="osb",
                                          name=f"osb_{qc}")
                sl = slice(dvh * DVH, (dvh + 1) * DVH)
                nc.vector.tensor_copy(s["osb"][:, sl], s[f"O{dvh}"])
                ov = o_view[qc]
                nc.sync.dma_start(out=ov[:, :, sl] if len(ov.shape) == 3
                                  else ov[:, sl], in_=s["osb"][:, sl])

            def emit_fill(qc, n, dvh=0):
                # keep-warm filler matmuls into a dummy O-tagged psum tile
                dmy = opool.tile([128, DVH], fp32, tag=f"O{dvh}",
                                 name=f"dmy{dvh}_{qc}")
                for _ in range(n):
                    nc.tensor.matmul(dmy, lhsT=kT[:, 0:128], rhs=kT[:, 0:DVH],
                                     start=True, stop=True)

            # PV(j) bursts run one head late in qtile j+1 (h1: d0a, h2:
            # d0b+evict, h3: d1a) with the d1 tail at h0 of qtile j+2 —
            # every burst has >=1 head of slack behind its AT transpose,
            # so the strict-FIFO PE queue never stalls at a qtile boundary.
            for qc in range(NQT):
                P = pwork.tile([128, H, NK], f16, tag="P", name=f"P_{qc}")
                racc = small.tile([128, H, 2], fp32, tag="racc",
                                  name=f"racc_{qc}")
                A = awork.tile([128, NK], f16, tag="A", name=f"A_{qc}")
                AT = awork.tile([128, NK], f16, tag="AT", name=f"AT_{qc}")
                st[qc] = {"P": P, "racc": racc, "A": A, "AT": AT}
                for h in range(H):
                    S0 = emit_scores(qc, h, 0)
                    S1 = emit_scores(qc, h, 1)
                    emit_exp(qc, h, 0, S0)
                    emit_exp(qc, h, 1, S1)
                    if h == 0:
                        if qc >= 2:
                            emit_pv(qc - 2, 1, range(8, NKT))
                            emit_evict_store(qc - 2, 1)
                    elif h == 1:
                        if qc >= 1:
                            emit_pv(qc - 1, 0, range(0, 8))
                        else:
                            emit_fill(qc, 8, dvh=0)
                    elif h == 2:
                        if qc >= 1:
                            emit_pv(qc - 1, 0, range(8, NKT))
                            emit_evict_store(qc - 1, 0)
                        else:
                            emit_fill(qc, 8, dvh=0)
                    elif h == 3:
                        if qc >= 1:
                            emit_pv(qc - 1, 1, range(0, 8))
                        else:
                            emit_fill(qc, 8, dvh=1)
                    emit_apath(qc, h)

            # ---- drain: d1 tail of qtile 6, then all of qtile 7 ----
            emit_pv(NQT - 2, 1, range(8, NKT))
            emit_evict_store(NQT - 2, 1)
            qc = NQT - 1
            emit_fill(qc, 6, dvh=0)  # keep PE warm while AT(7) lands
            emit_pv(qc, 0, range(0, 8))
            emit_pv(qc, 1, range(0, 8))
            emit_pv(qc, 0, range(8, NKT))
            emit_evict_store(qc, 0)
            emit_pv(qc, 1, range(8, NKT))
            emit_evict_store(qc, 1)


def _get_nc():
    if "nc" not in _CACHE:
        _CACHE["nc"] = _build_nc()
    return _CACHE["nc"]


def _host_prep(query, key, value, rms_q_w, rms_k_w, Wq, Wk, bq, bk):
    s = np.sqrt(float(HD))
    wqt = (rms_q_w[:, None] * Wq.T / s).astype(np.float16)
    wkt = (rms_k_w[:, None] * Wk.T).astype(np.float16)
    bqe = (bq / s).astype(np.float32)
    bke = bk.astype(np.float32)
    vq = (0.25 * value).astype(np.float16)  # [B, NK, DV]
    in_maps = []
    nq_blk = NQ // (N_CORES // B)  # 1024
    for c in range(N_CORES):
        b, qi = divmod(c, N_CORES // B)
        in_maps.append(
            {
                "q": np.ascontiguousarray(
                    query[b, qi * nq_blk : (qi + 1) * nq_blk]
                ).astype(np.float16),
                "k": np.ascontiguousarray(key[b]).astype(np.float16),
                "v": np.ascontiguousarray(vq[b]),
                "wqt": wqt,
                "wkt": wkt,
                "bqe": bqe,
                "bke": bke,
            }
        )
    return in_maps


def kernel(query, key, value, rms_q_w, rms_k_w, Wq, Wk, bq, bk, _trace=False):
    from concourse import bass_utils

    in_maps = _host_prep(
        np.asarray(query), np.asarray(key), np.asarray(value),
        np.asarray(rms_q_w), np.asarray(rms_k_w),
        np.asarray(Wq), np.asarray(Wk), np.asarray(bq), np.asarray(bk),
    )
    nc = _get_nc()
    res = bass_utils.run_bass_kernel_spmd(
        nc, in_maps, core_ids=list(range(N_CORES)), trace=_trace
    )
    _CACHE["last_results"] = res
    outs = [np.asarray(r["o"], dtype=np.float32) for r in res.results]
    nq_blk = NQ // (N_CORES // B)
    out = np.empty((B, NQ, DV), dtype=np.float32)
    for c in range(N_CORES):
        b, qi = divmod(c, N_CORES // B)
        out[b, qi * nq_blk : (qi + 1) * nq_blk] = outs[c]
    return out


# revision 15
# speedup vs baseline: 1.1695x; 1.0327x over previous
# Trainium2 Bass kernel for nn_CrossAttention (B=2, Nq=4096, Nk=2048, D=128,
# Dv=768, H=4, hd=32).
#
# Sharding: data-parallel over (B x Nq-blocks): core c handles batch c//4,
# query rows (c%4)*1024 .. +1024. K/V/weights replicated per core.
#
# Math (host-folded):
#   qn = (q * rstd_q) @ WqT_eff + bq/sqrt(hd)   with WqT_eff = diag(rms_q_w) Wq^T / sqrt(hd)
#   kn = (k * rstd_k) @ WkT_eff + bk            with WkT_eff = diag(rms_k_w) Wk^T
#   S_h = qn_h kn_h^T  (scale already folded into q side)
#   A   = sum_h exp(S_h) / rowsum_h(exp S_h)    (no max subtraction: |S| < 8)
#   out = A @ (0.25 * V)
#
# Pipeline: ACT(exp) is the pacer (~10.6us/qtile floor). lag-1 structure:
# per head, exp -> (rsum, recip, apath increment) on DVE immediately; PV of
# qtile qc-1 interleaved into PE gaps between score matmuls of qc.
import numpy as np

B, NQ, NK, D, DV = 2, 4096, 2048, 128, 768
H, HD = 4, 32
N_CORES = 8
NQC = NQ * B // N_CORES  # 1024 queries per core
NQT = NQC // 128  # 8 query tiles per core
NKT = NK // 128  # 16 key tiles
RMS_EPS = 1.1920929e-07
DVH = DV // 2  # 384: one PSUM bank per dv-half

_CACHE = {}


def _build_nc():
    import concourse.bacc as bacc
    import concourse.mybir as mybir
    import concourse.tile as tile

    fp32 = mybir.dt.float32
    f16 = mybir.dt.float16

    nc = bacc.Bacc("TRN2", target_bir_lowering=False, debug=False)

    q_d = nc.dram_tensor("q", [NQC, D], f16, kind="ExternalInput").ap()
    k_d = nc.dram_tensor("k", [NK, D], f16, kind="ExternalInput").ap()
    v_d = nc.dram_tensor("v", [NK, DV], f16, kind="ExternalInput").ap()
    wq_d = nc.dram_tensor("wqt", [D, D], f16, kind="ExternalInput").ap()
    wk_d = nc.dram_tensor("wkt", [D, D], f16, kind="ExternalInput").ap()
    bq_d = nc.dram_tensor("bqe", [D], fp32, kind="ExternalInput").ap()
    bk_d = nc.dram_tensor("bke", [D], fp32, kind="ExternalInput").ap()
    o_d = nc.dram_tensor("o", [NQC, DV], f16, kind="ExternalOutput").ap()

    with tile.TileContext(nc) as tc:
        _tile_kernel(tc, o_d, q_d, k_d, v_d, wq_d, wk_d, bq_d, bk_d)
    nc.compile()
    return nc


def _tile_kernel(tc, o_d, q_d, k_d, v_d, wq_d, wk_d, bq_d, bk_d):
    from contextlib import ExitStack

    import concourse.mybir as mybir

    nc = tc.nc
    fp32 = mybir.dt.float32
    f16 = mybir.dt.float16
    AF = mybir.ActivationFunctionType
    OP = mybir.AluOpType
    AX = mybir.AxisListType

    ctx = ExitStack()
    with ctx:
        singles = ctx.enter_context(tc.tile_pool(name="singles", bufs=1))

        # --- input loads. p-outer layouts: partition p reads a CONTIGUOUS
        # run of rows -> large DMA descriptors. Token order inside the core
        # becomes p-outer-permuted; consistent everywhere (kT<->V, qT<->out).
        # k in 2 halves so stats can start at half ready; q likewise.
        kx_sb = singles.tile([128, NKT, D], f16)
        nc.sync.dma_start(
            out=kx_sb[:, 0:8, :],
            in_=k_d.rearrange("(p c) d -> p c d", c=NKT)[:, 0:8, :],
        )
        nc.sync.dma_start(
            out=kx_sb[:, 8:16, :],
            in_=k_d.rearrange("(p c) d -> p c d", c=NKT)[:, 8:16, :],
        )
        qx_sb = singles.tile([128, NQT, D], f16)
        nc.sync.dma_start(
            out=qx_sb[:, 0:4, :],
            in_=q_d.rearrange("(p c) d -> p c d", c=NQT)[:, 0:4, :],
        )
        nc.sync.dma_start(
            out=qx_sb[:, 4:8, :],
            in_=q_d.rearrange("(p c) d -> p c d", c=NQT)[:, 4:8, :],
        )
        # v tile declared here but its DMA is dispatched late (after the
        # stats) so its 3MB stream never contends with k/q/w loads. Only
        # PV needs v (~20us in).
        v_sb = singles.tile([128, NKT, DV], f16)
        # weights/biases via gpsimd SWDGE (keeps sync queue clear for
        # transposes and ACT free for sqrt/exp); k-side first.
        wk_sb = singles.tile([128, D], f16)
        nc.gpsimd.dma_start(out=wk_sb, in_=wk_d)
        bk_sb = singles.tile([128, 1], fp32)
        nc.gpsimd.dma_start(out=bk_sb, in_=bk_d[:, None])
        wq_sb = singles.tile([128, D], f16)
        nc.gpsimd.dma_start(out=wq_sb, in_=wq_d)
        bq_sb = singles.tile([128, 1], fp32)
        nc.gpsimd.dma_start(out=bq_sb, in_=bq_d[:, None])

        eps_sb = singles.tile([128, 1], fp32)
        nc.vector.memset(eps_sb, RMS_EPS)
        warmsrc = singles.tile([128, 512], f16)
        nc.vector.memset(warmsrc, 0.125)

        kxT = singles.tile([128, NK], f16)  # normalized, transposed [d, tok]
        qxT = singles.tile([128, NQC], f16)
        kT = singles.tile([128, NK], f16)  # projected (head h rows 32h..32h+31)
        qT = singles.tile([128, NQC], f16)

        # ---- preamble: RMSNorm + transpose + projections (k first) ----
        with (
            tc.tile_pool(name="pre", bufs=1) as pre,
            tc.tile_pool(name="prepsum", bufs=2, space="PSUM") as prepsum,
        ):
            # PE warm-up at t~0: sustained matmul busy fires the HAM
            # un-throttle (1.2 -> 2.4 GHz) and bridges until proj arrives.
            warm = prepsum.tile([128, 512], fp32, tag="warm", bufs=1)
            for _ in range(16):
                nc.tensor.matmul(warm, lhsT=warmsrc[:, 0:128], rhs=warmsrc,
                                 start=True, stop=True)

            def mk_side(nt, tag):
                ssq = pre.tile([128, nt], fp32, tag=f"ssq{tag}", name=f"ssq{tag}")
                sd = pre.tile([128, nt], fp32, tag=f"sd{tag}", name=f"sd{tag}")
                rstd = pre.tile([128, nt], fp32, tag=f"rstd{tag}", name=f"rstd{tag}")
                xn = pre.tile([128, nt, D], f16, tag=f"xn{tag}", name=f"xn{tag}")
                return ssq, sd, rstd, xn

            def stats_half(x_sb, side, hh, n8=8):
                ssq, sd, rstd, xn = side
                sl = slice(hh * n8, (hh + 1) * n8)
                hsl = slice(hh * n8, (hh + 1) * n8)
                sq = pre.tile([128, n8, D], f16, tag="sqh", bufs=2,
                              name=f"sq_{id(side)}_{hh}")
                nc.vector.tensor_mul(sq, x_sb[:, sl, :], x_sb[:, sl, :])
                nc.vector.tensor_reduce(ssq[:, hsl, None], sq, AX.X, OP.add)
                nc.scalar.activation(
                    sd[:, hsl], ssq[:, hsl], AF.Sqrt, bias=eps_sb, scale=1.0 / D
                )
                nc.vector.reciprocal(rstd[:, hsl], sd[:, hsl])
                # one broadcast-mul (stride-0 inner dim) instead of n8
                # per-token tensor_scalar_muls: shorter DVE program
                nc.vector.tensor_mul(
                    xn[:, sl, :], x_sb[:, sl, :],
                    rstd[:, hsl, None].broadcast_to([128, n8, D]),
                )

            def transpose_half(xn, xT, hh, n8=8):
                w = n8 * 128
                nc.sync.dma_start_transpose(
                    out=xT[:, hh * w : (hh + 1) * w].rearrange(
                        "p (c j) -> p c j", j=128
                    ),
                    in_=xn[:, hh * n8 : (hh + 1) * n8, :].rearrange(
                        "p c j -> p (c j)"
                    ),
                )

            _pj = [0]

            def proj(xT, w_sb, b_sb, dst, j):
                # bias-add eviction on ACT (Identity w/ per-partition bias);
                # keeps DVE free for stats of the next half.
                _pj[0] += 1
                pp = prepsum.tile([128, 512], fp32, tag="proj", name=f"pp{_pj[0]}")
                nc.tensor.matmul(
                    pp, lhsT=w_sb, rhs=xT[:, j * 512 : (j + 1) * 512],
                    start=True, stop=True,
                )
                nc.scalar.activation(
                    dst[:, j * 512 : (j + 1) * 512], pp, AF.Identity, bias=b_sb
                )

            kside = mk_side(NKT, "k")
            qside = mk_side(NQT, "q")
            kxn, qxn = kside[3], qside[3]
            # k-side chain first (kT gates the first scores); q chunk 0
            # next; q chunk 1 is needed only from qtile 4 (~60us in).
            stats_half(kx_sb, kside, 0)
            transpose_half(kxn, kxT, 0)
            proj(kxT, wk_sb, bk_sb, kT, 0)
            proj(kxT, wk_sb, bk_sb, kT, 1)
            stats_half(kx_sb, kside, 1)
            transpose_half(kxn, kxT, 1)
            proj(kxT, wk_sb, bk_sb, kT, 2)
            proj(kxT, wk_sb, bk_sb, kT, 3)
            stats_half(qx_sb, qside, 0, n8=4)
            transpose_half(qxn, qxT, 0, n8=4)
            proj(qxT, wq_sb, bq_sb, qT, 0)
            stats_half(qx_sb, qside, 1, n8=4)
            # v load dispatched only now: its 3MB stream stays clear of the
            # k/q/w critical window. PV first needs it ~20us in.
            nc.scalar.dma_start(
                out=v_sb, in_=v_d.rearrange("(p c) d -> p c d", c=NKT)
            )
            # preload the exp table set (all Sqrt calls are emitted above,
            # so no further table swap happens before the real exps).
            dummy_exp = pre.tile([128, 1], fp32, tag="dexp", bufs=1)
            nc.scalar.activation(dummy_exp, eps_sb, AF.Exp)
            transpose_half(qxn, qxT, 1, n8=4)
            proj(qxT, wq_sb, bq_sb, qT, 1)

        # ---- main loop: lag-1 software pipeline ----
        with (
            tc.tile_pool(name="spsum", bufs=3, space="PSUM") as spool,
            tc.tile_pool(name="opsum", bufs=1, space="PSUM") as opool,
            tc.tile_pool(name="pwork", bufs=2) as pwork,
            tc.tile_pool(name="awork", bufs=2) as awork,
            tc.tile_pool(name="twork", bufs=2) as twork,
            tc.tile_pool(name="owork", bufs=2) as owork,
            tc.tile_pool(name="small", bufs=2) as small,
        ):
            st = {}
            o_view = o_d.rearrange("(j c) d -> c j d", c=NQT)

            def emit_scores(qc, h, half):
                # S chunk [128,1024] fp32 (2 PSUM banks) via 2 FD-512 matmuls
                qsl = slice(qc * 128, (qc + 1) * 128)
                S = spool.tile([128, 1024], fp32, tag="S",
                               name=f"S_{qc}_{h}_{half}")
                for kc in range(2):
                    ko = half * 1024 + kc * 512
                    nc.tensor.matmul(
                        S[:, kc * 512 : (kc + 1) * 512],
                        lhsT=qT[32 * h : 32 * (h + 1), qsl],
                        rhs=kT[32 * h : 32 * (h + 1), ko : ko + 512],
                        start=True, stop=True,
                        tile_position=(32 * h, 0),
                    )
                return S

            def emit_exp(qc, h, half, S):
                s = st[qc]
                psl = s["P"][:, h, half * 1024 : (half + 1) * 1024]
                nc.scalar.activation(
                    psl, S, AF.Exp, accum_out=s["racc"][:, h, half : half + 1]
                )

            def emit_at(qc, c0, c1):
                # transpose A cols [c0*128, c1*128) into AT
                s = st[qc]
                ksl = slice(c0 * 128, c1 * 128)
                nc.sync.dma_start_transpose(
                    out=s["AT"][:, ksl].rearrange("p (c j) -> p c j", j=128),
                    in_=s["A"][:, ksl],
                )

            def emit_apath(qc, h):
                # after exp(qc,h,1): crec_h then A += crec_h * P_h (2 halves);
                # for h==3 each completed A-half immediately launches its
                # transpose so PV(qc) can start early next qtile.
                s = st[qc]
                rsum = small.tile([128, 1], fp32, tag=f"rs{h}", name=f"rs_{qc}_{h}")
                nc.vector.tensor_scalar(
                    rsum, s["racc"][:, h, 0:1], s["racc"][:, h, 1:2], None, OP.add
                )
                crec = small.tile([128, 1], fp32, tag=f"cr{h}", name=f"cr_{qc}_{h}")
                nc.vector.reciprocal(crec, rsum)
                A, P = s["A"], s["P"]
                for half in range(2):
                    hsl = slice(half * 1024, (half + 1) * 1024)
                    if h == 0:
                        nc.vector.tensor_scalar_mul(A[:, hsl], P[:, 0, hsl], crec)
                    else:
                        t = twork.tile([128, 1024], f16, tag=f"t{half}",
                                       name=f"t_{qc}_{h}_{half}")
                        nc.vector.tensor_scalar_mul(t, P[:, h, hsl], crec)
                        nc.vector.tensor_add(A[:, hsl], A[:, hsl], t)
                    if h == 3:
                        if qc == NQT - 1:
                            # quarter-granular for the drain pipeline
                            emit_at(qc, half * 8, half * 8 + 4)
                            emit_at(qc, half * 8 + 4, half * 8 + 8)
                        else:
                            emit_at(qc, half * 8, half * 8 + 8)

            def emit_pv(qc, dvh, kcs):
                s = st[qc]
                key = f"O{dvh}"
                if key not in s:
                    s[key] = opool.tile([128, DVH], fp32, tag=key,
                                        name=f"{key}_{qc}")
                O, AT = s[key], s["AT"]
                for kc in kcs:
                    nc.tensor.matmul(
                        O,
                        lhsT=AT[:, kc * 128 : (kc + 1) * 128],
                        rhs=v_sb[:, kc, dvh * DVH : (dvh + 1) * DVH],
                        start=kc == 0,
                        stop=kc == NKT - 1,
                    )

            def emit_evict_store(qc, dvh):
                s = st[qc]
                if "osb" not in s:
                    s["osb"] = owork.tile([128, DV], f16, tag="osb",
                                          name=f"osb_{qc}")
                sl = slice(dvh * DVH, (dvh + 1) * DVH)
                nc.vector.tensor_copy(s["osb"][:, sl], s[f"O{dvh}"])
                ov = o_view[qc]
                nc.sync.dma_start(out=ov[:, :, sl] if len(ov.shape) == 3
                                  else ov[:, sl], in_=s["osb"][:, sl])

            def emit_fill(qc, n, dvh=0):
                # keep-warm filler matmuls into a dummy O-tagged psum tile
                dmy = opool.tile([128, DVH], fp32, tag=f"O{dvh}",
                                 name=f"dmy{dvh}_{qc}")
                for _ in range(n):
                    nc.tensor.matmul(dmy, lhsT=kT[:, 0:128], rhs=kT[:, 0:DVH],
                                     start=True, stop=True)

            # PV(j) bursts run one head late in qtile j+1 (h1: d0a, h2:
            # d0b+evict, h3: d1a) with the d1 tail at h0 of qtile j+2 —
            # every burst has >=1 head of slack behind its AT transpose,
            # so the strict-FIFO PE queue never stalls at a qtile boundary.
            for qc in range(NQT):
                P = pwork.tile([128, H, NK], f16, tag="P", name=f"P_{qc}")
                racc = small.tile([128, H, 2], fp32, tag="racc",
                                  name=f"racc_{qc}")
                A = awork.tile([128, NK], f16, tag="A", name=f"A_{qc}")
                AT = awork.tile([128, NK], f16, tag="AT", name=f"AT_{qc}")
                st[qc] = {"P": P, "racc": racc, "A": A, "AT": AT}
                for h in range(H):
                    S0 = emit_scores(qc, h, 0)
                    S1 = emit_scores(qc, h, 1)
                    emit_exp(qc, h, 0, S0)
                    emit_exp(qc, h, 1, S1)
                    if h == 0:
                        if qc >= 2:
                            emit_pv(qc - 2, 1, range(8, NKT))
                            emit_evict_store(qc - 2, 1)
                    elif h == 1:
                        if qc >= 1:
                            emit_pv(qc - 1, 0, range(0, 8))
                        else:
                            emit_fill(qc, 8, dvh=0)
                    elif h == 2:
                        if qc >= 1:
                            emit_pv(qc - 1, 0, range(8, NKT))
                            emit_evict_store(qc - 1, 0)
                        else:
                            emit_fill(qc, 8, dvh=0)
                    elif h == 3:
                        if qc >= 1:
                            emit_pv(qc - 1, 1, range(0, 8))
                        else:
                            emit_fill(qc, 8, dvh=1)
                    emit_apath(qc, h)

            # ---- drain: d1 tail of qtile 6, then all of qtile 7 ----
            emit_pv(NQT - 2, 1, range(8, NKT))
            emit_evict_store(NQT - 2, 1)
            qc = NQT - 1
            emit_fill(qc, 14, dvh=0)  # bridge PE over the AT(7) latency
            for qtr in range(4):
                kcs = range(4 * qtr, 4 * qtr + 4)
                emit_pv(qc, 0, kcs)
                emit_pv(qc, 1, kcs)
            emit_evict_store(qc, 0)
            emit_evict_store(qc, 1)


def _get_nc():
    if "nc" not in _CACHE:
        _CACHE["nc"] = _build_nc()
    return _CACHE["nc"]


def _host_prep(query, key, value, rms_q_w, rms_k_w, Wq, Wk, bq, bk):
    s = np.sqrt(float(HD))
    wqt = (rms_q_w[:, None] * Wq.T / s).astype(np.float16)
    wkt = (rms_k_w[:, None] * Wk.T).astype(np.float16)
    bqe = (bq / s).astype(np.float32)
    bke = bk.astype(np.float32)
    vq = (0.25 * value).astype(np.float16)  # [B, NK, DV]
    in_maps = []
    nq_blk = NQ // (N_CORES // B)  # 1024
    for c in range(N_CORES):
        b, qi = divmod(c, N_CORES // B)
        in_maps.append(
            {
                "q": np.ascontiguousarray(
                    query[b, qi * nq_blk : (qi + 1) * nq_blk]
                ).astype(np.float16),
                "k": np.ascontiguousarray(key[b]).astype(np.float16),
                "v": np.ascontiguousarray(vq[b]),
                "wqt": wqt,
                "wkt": wkt,
                "bqe": bqe,
                "bke": bke,
            }
        )
    return in_maps


def kernel(query, key, value, rms_q_w, rms_k_w, Wq, Wk, bq, bk, _trace=False):
    from concourse import bass_utils

    in_maps = _host_prep(
        np.asarray(query), np.asarray(key), np.asarray(value),
        np.asarray(rms_q_w), np.asarray(rms_k_w),
        np.asarray(Wq), np.asarray(Wk), np.asarray(bq), np.asarray(bk),
    )
    nc = _get_nc()
    res = bass_utils.run_bass_kernel_spmd(
        nc, in_maps, core_ids=list(range(N_CORES)), trace=_trace
    )
    _CACHE["last_results"] = res
    outs = [np.asarray(r["o"], dtype=np.float32) for r in res.results]
    nq_blk = NQ // (N_CORES // B)
    out = np.empty((B, NQ, DV), dtype=np.float32)
    for c in range(N_CORES):
        b, qi = divmod(c, N_CORES // B)
        out[b, qi * nq_blk : (qi + 1) * nq_blk] = outs[c]
    return out


# revision 18
# speedup vs baseline: 1.2299x; 1.0517x over previous
# Trainium2 Bass kernel for nn_CrossAttention (B=2, Nq=4096, Nk=2048, D=128,
# Dv=768, H=4, hd=32).
#
# Sharding: data-parallel over (B x Nq-blocks): core c handles batch c//4,
# query rows (c%4)*1024 .. +1024. K/V/weights replicated per core.
#
# Math (host-folded):
#   qn = (q * rstd_q) @ WqT_eff + bq/sqrt(hd)   with WqT_eff = diag(rms_q_w) Wq^T / sqrt(hd)
#   kn = (k * rstd_k) @ WkT_eff + bk            with WkT_eff = diag(rms_k_w) Wk^T
#   S_h = qn_h kn_h^T  (scale already folded into q side)
#   A   = sum_h exp(S_h) / rowsum_h(exp S_h)    (no max subtraction: |S| < 8)
#   out = A @ (0.25 * V)
#
# Pipeline: ACT(exp) is the pacer (~10.6us/qtile floor). lag-1 structure:
# per head, exp -> (rsum, recip, apath increment) on DVE immediately; PV of
# qtile qc-1 interleaved into PE gaps between score matmuls of qc.
import numpy as np

B, NQ, NK, D, DV = 2, 4096, 2048, 128, 768
H, HD = 4, 32
N_CORES = 8
NQC = NQ * B // N_CORES  # 1024 queries per core
NQT = NQC // 128  # 8 query tiles per core
NKT = NK // 128  # 16 key tiles
RMS_EPS = 1.1920929e-07
DVH = DV // 2  # 384: one PSUM bank per dv-half

_CACHE = {}


def _build_nc():
    import concourse.bacc as bacc
    import concourse.mybir as mybir
    import concourse.tile as tile

    fp32 = mybir.dt.float32
    f16 = mybir.dt.float16

    nc = bacc.Bacc("TRN2", target_bir_lowering=False, debug=False)

    q_d = nc.dram_tensor("q", [NQC, D], f16, kind="ExternalInput").ap()
    k_d = nc.dram_tensor("k", [NK, D], f16, kind="ExternalInput").ap()
    v_d = nc.dram_tensor("v", [NK, DV], f16, kind="ExternalInput").ap()
    wq_d = nc.dram_tensor("wqt", [D, D], f16, kind="ExternalInput").ap()
    wk_d = nc.dram_tensor("wkt", [D, D], f16, kind="ExternalInput").ap()
    bq_d = nc.dram_tensor("bqe", [D], fp32, kind="ExternalInput").ap()
    bk_d = nc.dram_tensor("bke", [D], fp32, kind="ExternalInput").ap()
    o_d = nc.dram_tensor("o", [NQC, DV], f16, kind="ExternalOutput").ap()

    with tile.TileContext(nc) as tc:
        _tile_kernel(tc, o_d, q_d, k_d, v_d, wq_d, wk_d, bq_d, bk_d)
    nc.compile()
    return nc


def _tile_kernel(tc, o_d, q_d, k_d, v_d, wq_d, wk_d, bq_d, bk_d):
    from contextlib import ExitStack

    import concourse.mybir as mybir

    nc = tc.nc
    fp32 = mybir.dt.float32
    f16 = mybir.dt.float16
    AF = mybir.ActivationFunctionType
    OP = mybir.AluOpType
    AX = mybir.AxisListType

    ctx = ExitStack()
    with ctx:
        singles = ctx.enter_context(tc.tile_pool(name="singles", bufs=1))

        # --- input loads. p-outer layouts: partition p reads a CONTIGUOUS
        # run of rows -> large DMA descriptors. Token order inside the core
        # becomes p-outer-permuted; consistent everywhere (kT<->V, qT<->out).
        # k in 2 halves so stats can start at half ready; q likewise.
        kx_sb = singles.tile([128, NKT, D], f16)
        nc.sync.dma_start(
            out=kx_sb[:, 0:8, :],
            in_=k_d.rearrange("(p c) d -> p c d", c=NKT)[:, 0:8, :],
        )
        nc.sync.dma_start(
            out=kx_sb[:, 8:16, :],
            in_=k_d.rearrange("(p c) d -> p c d", c=NKT)[:, 8:16, :],
        )
        qx_sb = singles.tile([128, NQT, D], f16)
        nc.sync.dma_start(
            out=qx_sb[:, 0:4, :],
            in_=q_d.rearrange("(p c) d -> p c d", c=NQT)[:, 0:4, :],
        )
        nc.sync.dma_start(
            out=qx_sb[:, 4:8, :],
            in_=q_d.rearrange("(p c) d -> p c d", c=NQT)[:, 4:8, :],
        )
        # v tile declared here but its DMA is dispatched late (after the
        # stats) so its 3MB stream never contends with k/q/w loads. Only
        # PV needs v (~20us in).
        v_sb = singles.tile([128, NKT, DV], f16)
        # weights/biases via gpsimd SWDGE (keeps sync queue clear for
        # transposes and ACT free for sqrt/exp); k-side first.
        wk_sb = singles.tile([128, D], f16)
        nc.gpsimd.dma_start(out=wk_sb, in_=wk_d)
        bk_sb = singles.tile([128, 1], fp32)
        nc.gpsimd.dma_start(out=bk_sb, in_=bk_d[:, None])
        wq_sb = singles.tile([128, D], f16)
        nc.gpsimd.dma_start(out=wq_sb, in_=wq_d)
        bq_sb = singles.tile([128, 1], fp32)
        nc.gpsimd.dma_start(out=bq_sb, in_=bq_d[:, None])
        # v LAST on the gpsimd SWDGE queue: the scheduler keeps same-engine
        # emission order for equally-ready DMAs, so v's 3MB stream starts
        # only after k/q/w descriptors are in flight. PV needs v ~20us in.
        nc.gpsimd.dma_start(
            out=v_sb, in_=v_d.rearrange("(p c) d -> p c d", c=NKT)
        )

        eps_sb = singles.tile([128, 1], fp32)
        nc.vector.memset(eps_sb, RMS_EPS)
        warmsrc = singles.tile([128, 512], f16)
        nc.vector.memset(warmsrc, 0.125)

        kxT = singles.tile([128, NK], f16)  # normalized, transposed [d, tok]
        qxT = singles.tile([128, NQC], f16)
        kT = singles.tile([128, NK], f16)  # projected (head h rows 32h..32h+31)
        qT = singles.tile([128, NQC], f16)

        # ---- preamble: RMSNorm + transpose + projections (k first) ----
        with (
            tc.tile_pool(name="pre", bufs=1) as pre,
            tc.tile_pool(name="prepsum", bufs=3, space="PSUM") as prepsum,
        ):
            # PE warm-up at t~0: sustained matmul busy fires the HAM
            # un-throttle (1.2 -> 2.4 GHz) and bridges until proj arrives.
            warm = prepsum.tile([128, 512], fp32, tag="warm", bufs=1)
            for _ in range(16):
                nc.tensor.matmul(warm, lhsT=warmsrc[:, 0:128], rhs=warmsrc,
                                 start=True, stop=True)

            def mk_side(nt, tag):
                ssq = pre.tile([128, nt], fp32, tag=f"ssq{tag}", name=f"ssq{tag}")
                sd = pre.tile([128, nt], fp32, tag=f"sd{tag}", name=f"sd{tag}")
                rstd = pre.tile([128, nt], fp32, tag=f"rstd{tag}", name=f"rstd{tag}")
                xn = pre.tile([128, nt, D], f16, tag=f"xn{tag}", name=f"xn{tag}")
                return ssq, sd, rstd, xn

            def stats_half(x_sb, side, hh, n8=8):
                ssq, sd, rstd, xn = side
                sl = slice(hh * n8, (hh + 1) * n8)
                hsl = slice(hh * n8, (hh + 1) * n8)
                sq = pre.tile([128, n8, D], f16, tag="sqh", bufs=2,
                              name=f"sq_{id(side)}_{hh}")
                nc.vector.tensor_mul(sq, x_sb[:, sl, :], x_sb[:, sl, :])
                nc.vector.tensor_reduce(ssq[:, hsl, None], sq, AX.X, OP.add)
                nc.scalar.activation(
                    sd[:, hsl], ssq[:, hsl], AF.Sqrt, bias=eps_sb, scale=1.0 / D
                )
                nc.vector.reciprocal(rstd[:, hsl], sd[:, hsl])
                # one broadcast-mul (stride-0 inner dim) instead of n8
                # per-token tensor_scalar_muls: shorter DVE program
                nc.vector.tensor_mul(
                    xn[:, sl, :], x_sb[:, sl, :],
                    rstd[:, hsl, None].broadcast_to([128, n8, D]),
                )

            def transpose_half(xn, xT, hh, n8=8):
                w = n8 * 128
                nc.sync.dma_start_transpose(
                    out=xT[:, hh * w : (hh + 1) * w].rearrange(
                        "p (c j) -> p c j", j=128
                    ),
                    in_=xn[:, hh * n8 : (hh + 1) * n8, :].rearrange(
                        "p c j -> p (c j)"
                    ),
                )

            _pj = [0]

            def proj(xT, w_sb, b_sb, dst, j):
                # bias-add eviction on ACT (Identity w/ per-partition bias);
                # keeps DVE free for stats of the next half.
                _pj[0] += 1
                pp = prepsum.tile([128, 512], fp32, tag="proj", name=f"pp{_pj[0]}")
                nc.tensor.matmul(
                    pp, lhsT=w_sb, rhs=xT[:, j * 512 : (j + 1) * 512],
                    start=True, stop=True,
                )
                nc.scalar.activation(
                    dst[:, j * 512 : (j + 1) * 512], pp, AF.Identity, bias=b_sb
                )

            kside = mk_side(NKT, "k")
            qside = mk_side(NQT, "q")
            kxn, qxn = kside[3], qside[3]
            # k-side chain first (kT gates the first scores); q chunk 0
            # next; q chunk 1 is needed only from qtile 4 (~60us in).
            stats_half(kx_sb, kside, 0)
            transpose_half(kxn, kxT, 0)
            proj(kxT, wk_sb, bk_sb, kT, 0)
            proj(kxT, wk_sb, bk_sb, kT, 1)
            stats_half(kx_sb, kside, 1)
            transpose_half(kxn, kxT, 1)
            proj(kxT, wk_sb, bk_sb, kT, 2)
            proj(kxT, wk_sb, bk_sb, kT, 3)
            stats_half(qx_sb, qside, 0, n8=4)
            transpose_half(qxn, qxT, 0, n8=4)
            proj(qxT, wq_sb, bq_sb, qT, 0)
            stats_half(qx_sb, qside, 1, n8=4)
            # preload the exp table set (all Sqrt calls are emitted above,
            # so no further table swap happens before the real exps).
            dummy_exp = pre.tile([128, 1], fp32, tag="dexp", bufs=1)
            nc.scalar.activation(dummy_exp, eps_sb, AF.Exp)
            transpose_half(qxn, qxT, 1, n8=4)
            proj(qxT, wq_sb, bq_sb, qT, 1)

        # ---- main loop: lag-1 software pipeline ----
        with (
            tc.tile_pool(name="spsum", bufs=3, space="PSUM") as spool,
            tc.tile_pool(name="opsum", bufs=1, space="PSUM") as opool,
            tc.tile_pool(name="pwork", bufs=2) as pwork,
            tc.tile_pool(name="awork", bufs=2) as awork,
            tc.tile_pool(name="twork", bufs=2) as twork,
            tc.tile_pool(name="owork", bufs=2) as owork,
            tc.tile_pool(name="small", bufs=2) as small,
        ):
            st = {}
            o_view = o_d.rearrange("(j c) d -> c j d", c=NQT)

            def emit_scores(qc, h, half):
                # S chunk [128,1024] fp32 (2 PSUM banks) via 2 FD-512 matmuls
                qsl = slice(qc * 128, (qc + 1) * 128)
                S = spool.tile([128, 1024], fp32, tag="S",
                               name=f"S_{qc}_{h}_{half}")
                for kc in range(2):
                    ko = half * 1024 + kc * 512
                    nc.tensor.matmul(
                        S[:, kc * 512 : (kc + 1) * 512],
                        lhsT=qT[32 * h : 32 * (h + 1), qsl],
                        rhs=kT[32 * h : 32 * (h + 1), ko : ko + 512],
                        start=True, stop=True,
                        tile_position=(32 * h, 0),
                    )
                return S

            def emit_exp(qc, h, half, S):
                s = st[qc]
                psl = s["P"][:, h, half * 1024 : (half + 1) * 1024]
                nc.scalar.activation(
                    psl, S, AF.Exp, accum_out=s["racc"][:, h, half : half + 1]
                )

            def emit_at(qc, c0, c1):
                # transpose A cols [c0*128, c1*128) into AT
                s = st[qc]
                ksl = slice(c0 * 128, c1 * 128)
                nc.sync.dma_start_transpose(
                    out=s["AT"][:, ksl].rearrange("p (c j) -> p c j", j=128),
                    in_=s["A"][:, ksl],
                )

            def emit_apath(qc, h):
                # after exp(qc,h,1): crec_h then A += crec_h * P_h (2 halves);
                # for h==3 each completed A-half immediately launches its
                # transpose so PV(qc) can start early next qtile.
                s = st[qc]
                rsum = small.tile([128, 1], fp32, tag=f"rs{h}", name=f"rs_{qc}_{h}")
                nc.vector.tensor_scalar(
                    rsum, s["racc"][:, h, 0:1], s["racc"][:, h, 1:2], None, OP.add
                )
                crec = small.tile([128, 1], fp32, tag=f"cr{h}", name=f"cr_{qc}_{h}")
                nc.vector.reciprocal(crec, rsum)
                A, P = s["A"], s["P"]
                for half in range(2):
                    hsl = slice(half * 1024, (half + 1) * 1024)
                    if h == 0:
                        nc.vector.tensor_scalar_mul(A[:, hsl], P[:, 0, hsl], crec)
                    else:
                        t = twork.tile([128, 1024], f16, tag=f"t{half}",
                                       name=f"t_{qc}_{h}_{half}")
                        nc.vector.tensor_scalar_mul(t, P[:, h, hsl], crec)
                        nc.vector.tensor_add(A[:, hsl], A[:, hsl], t)
                    if h == 3:
                        if qc == NQT - 1:
                            # quarter-granular for the drain pipeline
                            emit_at(qc, half * 8, half * 8 + 4)
                            emit_at(qc, half * 8 + 4, half * 8 + 8)
                        else:
                            emit_at(qc, half * 8, half * 8 + 8)

            def emit_pv(qc, dvh, kcs):
                s = st[qc]
                key = f"O{dvh}"
                if key not in s:
                    s[key] = opool.tile([128, DVH], fp32, tag=key,
                                        name=f"{key}_{qc}")
                O, AT = s[key], s["AT"]
                for kc in kcs:
                    nc.tensor.matmul(
                        O,
                        lhsT=AT[:, kc * 128 : (kc + 1) * 128],
                        rhs=v_sb[:, kc, dvh * DVH : (dvh + 1) * DVH],
                        start=kc == 0,
                        stop=kc == NKT - 1,
                    )

            def emit_evict_store(qc, dvh):
                s = st[qc]
                if "osb" not in s:
                    s["osb"] = owork.tile([128, DV], f16, tag="osb",
                                          name=f"osb_{qc}")
                sl = slice(dvh * DVH, (dvh + 1) * DVH)
                nc.vector.tensor_copy(s["osb"][:, sl], s[f"O{dvh}"])
                ov = o_view[qc]
                nc.sync.dma_start(out=ov[:, :, sl] if len(ov.shape) == 3
                                  else ov[:, sl], in_=s["osb"][:, sl])

            def emit_fill(qc, n, dvh=0):
                # keep-warm filler matmuls into a dummy O-tagged psum tile
                dmy = opool.tile([128, DVH], fp32, tag=f"O{dvh}",
                                 name=f"dmy{dvh}_{qc}")
                for _ in range(n):
                    nc.tensor.matmul(dmy, lhsT=kT[:, 0:128], rhs=kT[:, 0:DVH],
                                     start=True, stop=True)

            # PV(j) bursts run one head late in qtile j+1 (h1: d0a, h2:
            # d0b+evict, h3: d1a) with the d1 tail at h0 of qtile j+2 —
            # every burst has >=1 head of slack behind its AT transpose,
            # so the strict-FIFO PE queue never stalls at a qtile boundary.
            for qc in range(NQT):
                P = pwork.tile([128, H, NK], f16, tag="P", name=f"P_{qc}")
                racc = small.tile([128, H, 2], fp32, tag="racc",
                                  name=f"racc_{qc}")
                A = awork.tile([128, NK], f16, tag="A", name=f"A_{qc}")
                AT = awork.tile([128, NK], f16, tag="AT", name=f"AT_{qc}")
                st[qc] = {"P": P, "racc": racc, "A": A, "AT": AT}
                for h in range(H):
                    S0 = emit_scores(qc, h, 0)
                    S1 = emit_scores(qc, h, 1)
                    emit_exp(qc, h, 0, S0)
                    emit_exp(qc, h, 1, S1)
                    if h == 0:
                        if qc >= 2:
                            emit_pv(qc - 2, 1, range(8, NKT))
                            emit_evict_store(qc - 2, 1)
                    elif h == 1:
                        if qc >= 1:
                            emit_fill(qc, 3, dvh=1)
                            emit_pv(qc - 1, 0, range(0, 8))
                        else:
                            emit_fill(qc, 8, dvh=0)
                    elif h == 2:
                        if qc >= 1:
                            emit_pv(qc - 1, 0, range(8, NKT))
                            emit_evict_store(qc - 1, 0)
                        else:
                            emit_fill(qc, 8, dvh=0)
                    elif h == 3:
                        if qc >= 1:
                            emit_fill(qc, 3, dvh=0)
                            emit_pv(qc - 1, 1, range(0, 8))
                        else:
                            emit_fill(qc, 8, dvh=1)
                    emit_apath(qc, h)

            # ---- drain: d1 tail of qtile 6, then all of qtile 7 ----
            emit_pv(NQT - 2, 1, range(8, NKT))
            emit_evict_store(NQT - 2, 1)
            qc = NQT - 1
            emit_fill(qc, 14, dvh=0)  # bridge PE over the AT(7) latency
            for qtr in range(4):
                kcs = range(4 * qtr, 4 * qtr + 4)
                emit_pv(qc, 0, kcs)
                emit_pv(qc, 1, kcs)
            emit_evict_store(qc, 0)
            emit_evict_store(qc, 1)


def _get_nc():
    if "nc" not in _CACHE:
        _CACHE["nc"] = _build_nc()
    return _CACHE["nc"]


def _host_prep(query, key, value, rms_q_w, rms_k_w, Wq, Wk, bq, bk):
    s = np.sqrt(float(HD))
    wqt = (rms_q_w[:, None] * Wq.T / s).astype(np.float16)
    wkt = (rms_k_w[:, None] * Wk.T).astype(np.float16)
    bqe = (bq / s).astype(np.float32)
    bke = bk.astype(np.float32)
    vq = (0.25 * value).astype(np.float16)  # [B, NK, DV]
    in_maps = []
    nq_blk = NQ // (N_CORES // B)  # 1024
    for c in range(N_CORES):
        b, qi = divmod(c, N_CORES // B)
        in_maps.append(
            {
                "q": np.ascontiguousarray(
                    query[b, qi * nq_blk : (qi + 1) * nq_blk]
                ).astype(np.float16),
                "k": np.ascontiguousarray(key[b]).astype(np.float16),
                "v": np.ascontiguousarray(vq[b]),
                "wqt": wqt,
                "wkt": wkt,
                "bqe": bqe,
                "bke": bke,
            }
        )
    return in_maps


def kernel(query, key, value, rms_q_w, rms_k_w, Wq, Wk, bq, bk, _trace=False):
    from concourse import bass_utils

    in_maps = _host_prep(
        np.asarray(query), np.asarray(key), np.asarray(value),
        np.asarray(rms_q_w), np.asarray(rms_k_w),
        np.asarray(Wq), np.asarray(Wk), np.asarray(bq), np.asarray(bk),
    )
    nc = _get_nc()
    res = bass_utils.run_bass_kernel_spmd(
        nc, in_maps, core_ids=list(range(N_CORES)), trace=_trace
    )
    _CACHE["last_results"] = res
    outs = [np.asarray(r["o"], dtype=np.float32) for r in res.results]
    nq_blk = NQ // (N_CORES // B)
    out = np.empty((B, NQ, DV), dtype=np.float32)
    for c in range(N_CORES):
        b, qi = divmod(c, N_CORES // B)
        out[b, qi * nq_blk : (qi + 1) * nq_blk] = outs[c]
    return out
